# revision 50
# baseline (speedup 1.0000x reference)
"""Trainium2 Bass kernel for nn_Block_89361089561275 (dense transformer block).

Sharding: data-parallel over batch B=8 -> one batch element per NeuronCore.
No collectives. The whole block runs in "feature-transposed" layout
(features on SBUF partitions, tokens on the free dim), which makes every
matmul a natural lhsT/rhs pair with zero on-device transposes:

    xT [C, N] --LN1--> xnT --+--> qkT = wqk^T @ xn  [1536, N]   (q pre-scaled)
                             +--> v   = xn @ wv     [N, 768]    (token-major)
    per head h (heads 2j/2j+1 live at partition offsets 0/64 -> the K=64
    score matmuls auto-pack as 64x128 PE row tiles):
        ST[m, n] = k_h^T(d,m).T @ q_h^T(d,n)        (scores, transposed)
        P~ = exp(ST + rpb^T)                         (no max-subtraction;
                                                      scores are O(1) here)
        [colsum; o~^T] = [ones; v_h]^T @ P~          (ones column makes the
                                                      softmax denominator fall
                                                      out of the AV matmul)
        o^T = o~^T * (1/colsum)  broadcast over d
    x1 = x + gamma1 * (proj_w^T-matmul + b);  LN2;  MLP + adapter fused into
    one PSUM accumulation; out = x1 + gamma2 * (...).

Weights are transposed/packed/cast to bf16 on the host (layout prep only);
all matmul accumulation is fp32 in PSUM.
"""

import sys

for _p in ("/opt/trn_rl_repo",):
    if _p not in sys.path:
        sys.path.insert(0, _p)

import numpy as np
import ml_dtypes

BF16 = ml_dtypes.bfloat16

B, N, C, H = 8, 1024, 768, 12
D = C // H            # 64
MLP = 4 * C           # 3072
RED = C // 3          # 256
EPS = 1e-5
P = 128
KC = C // P           # 6   c-chunks
KM = MLP // P         # 24  mlp-chunks
KR = RED // P         # 2   adapter chunks
NT = N // P           # 8   token tiles
HALF = 512
NSL = (slice(0, HALF), slice(HALF, N))

_PROG_CACHE: dict = {}

# indices into the packed [n, 128, KC] per-feature vector table
V_G1, V_G2, V_QB, V_PB, V_FB, V_L1G, V_L1B, V_L2G, V_L2B = range(9)
NVEC = 9


def _build(flags):
    """Build the single-core Bass program. flags is a tuple of bools:
    (has_mask, qb_nz, vb_nz, pb_nz, f1b_nz, fb_nz, adb_nz,
     ln1_gb_triv, ln2_gb_triv)
    """
    (has_mask, qb_nz, vb_nz, pb_nz, f1b_nz, fb_nz, adb_nz,
     ln1_triv, ln2_triv) = flags

    import concourse.tile as tile
    from concourse import bacc, mybir
    from contextlib import ExitStack

    f32 = mybir.dt.float32
    bf16 = mybir.dt.bfloat16
    AF = mybir.ActivationFunctionType
    OP = mybir.AluOpType

    nc = bacc.Bacc("TRN2")

    # ---- external I/O ----
    x_d = nc.declare_dram_parameter("xT", [P, KC, N], f32, isOutput=False)
    rpb_d = nc.declare_dram_parameter("rpbT", [H, N, N], bf16, isOutput=False)
    wqk_d = nc.declare_dram_parameter("wqk", [12, P, KC, P], bf16, isOutput=False)
    wv_d = nc.declare_dram_parameter("wv", [P, KC, C], bf16, isOutput=False)
    pw_d = nc.declare_dram_parameter("projw", [KC, P, KC, P], bf16, isOutput=False)
    f1_d = nc.declare_dram_parameter("fc1w", [KM, P, KC, P], bf16, isOutput=False)
    f2_d = nc.declare_dram_parameter("fc2w", [KC, P, KM, P], bf16, isOutput=False)
    ad_d = nc.declare_dram_parameter("adw", [KR, P, KC, P], bf16, isOutput=False)
    au_d = nc.declare_dram_parameter("auw", [KC, P, KR, P], bf16, isOutput=False)
    vec_d = nc.declare_dram_parameter("vecs", [NVEC, P, KC], f32, isOutput=False)
    f1b_d = nc.declare_dram_parameter("fc1b", [P, KM], f32, isOutput=False)
    adb_d = nc.declare_dram_parameter("adb", [P, KR], f32, isOutput=False)
    vb_d = nc.declare_dram_parameter("vbias", [1, C], f32, isOutput=False)
    mb_d = nc.declare_dram_parameter("maskb", [P, NT], f32, isOutput=False)
    out_d = nc.declare_dram_parameter("outT", [P, KC, N], f32, isOutput=True)

    with tile.TileContext(nc) as tc, ExitStack() as ctx:
        sb = ctx.enter_context(tc.tile_pool(name="sb", bufs=1))
        pp = ctx.enter_context(tc.tile_pool(name="pp", bufs=1, space="PSUM"))
        dram = ctx.enter_context(tc.tile_pool(name="dram", bufs=2, space="DRAM"))

        def broadcast(dst, src):
            """dst [p, n] sbuf <- src [1, n] sbuf replicated across partitions
            (via a DRAM bounce; SBUF APs cannot broadcast the partition dim)."""
            scratch = dram.tile([1, src.shape[-1]], src.dtype,
                                tag="bscratch", bufs=2, name="bscratch")
            nc.sync.dma_start(out=scratch, in_=src)
            nc.sync.dma_start(out=dst, in_=scratch.to_broadcast(dst.shape))

        def bcast_from_p128(dst, src128):
            """dst [p, n] sbuf <- broadcast of an [128, n/128] scattered row
            (element n lives at src128[n // (n/128), n % ...] p-major order)."""
            n = dst.shape[-1]
            scratch = dram.tile([1, n], src128.dtype,
                                tag="bscratch", bufs=2, name="bscratch")
            nc.sync.dma_start(out=scratch, in_=src128)
            nc.sync.dma_start(out=dst, in_=scratch.to_broadcast(dst.shape))

        # ---- persistent tiles ----
        xres = sb.tile([P, KC, N], f32, tag="xres", bufs=1)
        qkT = sb.tile([P, 12, N], bf16, tag="qkT", bufs=1)
        vaug = sb.tile([P, NT, H, D + 1], bf16, tag="vaug", bufs=1)
        ones_bf = sb.tile([P, 1], bf16, tag="ones", bufs=1)

        for ch in range(KC):  # per-chunk loads so LN1 stats start early
            nc.sync.dma_start(out=xres[:, ch], in_=x_d[:, ch])
        nc.vector.memset(ones_bf, 1.0)
        nc.vector.memset(vaug, 1.0)

        zero_col = sb.tile([P, 1], f32, tag="zcol", bufs=1)
        nc.vector.memset(zero_col, 0.0)
        eps_col = sb.tile([P, 1], f32, tag="ecol", bufs=1)
        nc.vector.memset(eps_col, float(EPS))

        vecs = sb.tile([P, NVEC, KC], f32, tag="vecs", bufs=1)
        nc.sync.dma_start(out=vecs, in_=vec_d[:].rearrange("v p k -> p v k"))

        def vec(i):
            return vecs[:, i]  # [128, KC]

        if f1b_nz:
            f1b = sb.tile([P, KM], f32, tag="f1b", bufs=1)
            nc.sync.dma_start(out=f1b, in_=f1b_d[:])
        if adb_nz:
            adb = sb.tile([P, KR], f32, tag="adb", bufs=1)
            nc.sync.dma_start(out=adb, in_=adb_d[:])
        if vb_nz:
            vb1 = sb.tile([1, C], f32, tag="vb1", bufs=1)
            nc.sync.dma_start(out=vb1, in_=vb_d[:])
            vb_b = sb.tile([P, C], f32, tag="vb_b", bufs=1)
            broadcast(vb_b, vb1)
        if has_mask:
            maskb = sb.tile([P, NT], f32, tag="maskb", bufs=1)
            nc.sync.dma_start(out=maskb, in_=mb_d[:])

        # ---------------- layernorm (feature-transposed) ----------------
        NP8 = N // P  # 8 — scattered-stat free width

        # LN stats live in one "st"-tag PSUM tile (idle outside attention):
        # sum(x) accumulates on row 0, sum(x^2) on row 32 (legal matmul
        # output base partitions), each token half on its own bank.
        def ln_stats_alloc():
            return pp.tile([33, N], f32, tag="st", bufs=2, name="ln_stats")

        def ln_stats_ch(stt, ch, xbs):
            xb = sb.tile([P, N], bf16, tag="xb", bufs=KC)
            xbs.append(xb)
            nc.vector.tensor_copy(out=xb, in_=xres[:, ch])
            x2 = sb.tile([P, N], bf16, tag="rpb", bufs=4)
            nc.vector.tensor_mul(x2, xb, xb)
            for nk in range(2):
                nc.tensor.matmul(stt[0:1, NSL[nk]], lhsT=ones_bf,
                                 rhs=xb[:, NSL[nk]],
                                 start=(ch == 0), stop=(ch == KC - 1))
                nc.tensor.matmul(stt[32:33, NSL[nk]], lhsT=ones_bf,
                                 rhs=x2[:, NSL[nk]],
                                 start=(ch == 0), stop=(ch == KC - 1))

        def ln_finish(stt, xbs, dst, g_i, b_i, triv):
            stat2 = sb.tile([33, N], f32, tag="stat", bufs=3)
            nc.vector.tensor_scalar_mul(stat2[0:1, :], stt[0:1, :], 1.0 / C)
            nc.vector.tensor_scalar_mul(stat2[32:33, :], stt[32:33, :],
                                        1.0 / C)
            # scatter the per-token stats across 128 partitions so the
            # var/sqrt/reciprocal chain runs on all lanes instead of one
            m128 = sb.tile([P, NP8], f32, tag="cs128", bufs=4)
            q128 = sb.tile([P, NP8], f32, tag="cs128", bufs=4)
            nc.sync.dma_start(out=m128, in_=stat2[0:1, :])
            nc.sync.dma_start(out=q128, in_=stat2[32:33, :])
            t128 = sb.tile([P, NP8], f32, tag="cs128", bufs=4)
            r128 = sb.tile([P, NP8], f32, tag="cs128", bufs=4)
            nc.vector.tensor_mul(t128, m128, m128)
            nc.vector.tensor_sub(q128, q128, t128)       # var
            nc.scalar.activation(q128, q128, AF.Sqrt, bias=eps_col)
            nc.vector.reciprocal(r128, q128)             # rstd
            # m128 <- -mean*rstd
            nc.vector.scalar_tensor_tensor(out=m128, in0=m128, scalar=-1.0,
                                           in1=r128, op0=OP.mult, op1=OP.mult)
            r_bf = sb.tile([P, NP8], bf16, tag="cs_bf", bufs=2)
            n_bf = sb.tile([P, NP8], bf16, tag="cs_bf", bufs=2)
            nc.vector.tensor_copy(out=r_bf, in_=r128)
            nc.vector.tensor_copy(out=n_bf, in_=m128)
            a_b = sb.tile([P, N], bf16, tag="bcast", bufs=2)
            c_b = sb.tile([P, N], bf16, tag="bcast", bufs=2)
            bcast_from_p128(a_b, r_bf)
            bcast_from_p128(c_b, n_bf)
            for ch in range(KC):
                t1 = sb.tile([P, N], bf16, tag="rpb", bufs=4)
                nc.vector.tensor_mul(t1, xbs[ch], a_b)
                if triv:
                    nc.vector.tensor_add(dst[:, ch], t1, c_b)
                else:
                    nc.vector.tensor_add(t1, t1, c_b)
                    nc.vector.tensor_scalar(
                        out=dst[:, ch], in0=t1,
                        scalar1=vec(g_i)[:, ch:ch + 1],
                        scalar2=vec(b_i)[:, ch:ch + 1],
                        op0=OP.mult, op1=OP.add)

        def layernorm(dst, g_i, b_i, triv):
            stt = ln_stats_alloc()
            xbs = []
            for ch in range(KC):
                ln_stats_ch(stt, ch, xbs)
            ln_finish(stt, xbs, dst, g_i, b_i, triv)

        # ---------------- LN1 + QKV ----------------
        xnT = sb.tile([P, KC, N], bf16, tag="feat", bufs=2)
        layernorm(xnT, V_L1G, V_L1B, ln1_triv)

        # q/k blocks interleaved so attention head-pair j can start as soon
        # as blocks j (q) and 6+j (k) are done
        for blk in (0, 6, 1, 7, 2, 8, 3, 9, 4, 10, 5, 11):
            wt = sb.tile([P, KC, P], bf16, tag="w6", bufs=3)
            nc.sync.dma_start(out=wt, in_=wqk_d[blk])
            for nk in range(2):
                mm = pp.tile([P, HALF], f32, tag="acc", bufs=4)
                for ks in range(KC):
                    nc.tensor.matmul(mm, lhsT=wt[:, ks],
                                     rhs=xnT[:, ks, NSL[nk]],
                                     start=(ks == 0), stop=(ks == KC - 1))
                dst = qkT[:, blk, NSL[nk]]
                if blk < 6 and qb_nz:
                    nc.vector.tensor_scalar_add(dst, mm, vec(V_QB)[:, blk:blk + 1])
                else:
                    nc.scalar.copy(out=dst, in_=mm)

        wv_sb = sb.tile([P, KC, C], bf16, tag="wv", bufs=1)
        nc.sync.dma_start(out=wv_sb, in_=wv_d[:])
        for t in range(NT):
            for off, cw in ((0, HALF), (HALF, C - HALF)):
                mm = pp.tile([P, HALF], f32, tag="acc", bufs=4)
                for ks in range(KC):
                    nc.tensor.matmul(mm[:, :cw],
                                     lhsT=xnT[:, ks, t * P:(t + 1) * P],
                                     rhs=wv_sb[:, ks, off:off + cw],
                                     start=(ks == 0), stop=(ks == KC - 1))
                dst = vaug[:, t, off // D:(off + cw) // D, :D]
                src = mm[:, :cw].rearrange("p (h d) -> p h d", d=D)
                if vb_nz:
                    nc.vector.tensor_add(
                        dst, src,
                        vb_b[:, off:off + cw].rearrange("p (h d) -> p h d", d=D))
                else:
                    nc.vector.tensor_copy(out=dst, in_=src)

        # ---------------- attention ----------------
        # Head pairs (2j, 2j+1) live at partition offsets 0/64 of qkT block j,
        # so their K=64 score matmuls execute concurrently in distinct PE
        # row-groups (and one head's LDWEIGHTS hides under the other's MM).
        oT = sb.tile([P, KC, N], bf16, tag="feat", bufs=2)

        def evac_head(o_ps, hp, hh):
            # copy the unnormalized head out of PSUM right away (frees the
            # accumulator slots for the next pair), normalize asynchronously
            ou = sb.tile([D + 1, N], f32, tag="stat", bufs=3)
            for nk in range(2):
                # ACT does the psum evacuation (it's idle at pair boundaries;
                # DVE is busy with the normalize multiplies)
                nc.scalar.copy(out=ou[:, NSL[nk]], in_=o_ps[nk])
            # broadcast the raw colsum, then approx-reciprocal on all lanes
            # (the approx op's bit-twiddling seed is broken at base part 64)
            raw = sb.tile([P, N], f32, tag="rec", bufs=2)
            broadcast(raw, ou[D:D + 1, :])
            rb = sb.tile([P, N], f32, tag="bcast2", bufs=2)
            nc.vector.reciprocal_approx_fast(out=rb[0:D, :], in_=raw[0:D, :])
            ot_tmp = sb.tile([P, N], bf16, tag="ott", bufs=2)
            nc.vector.tensor_mul(ot_tmp[0:D, :], ou[0:D, :], rb[0:D, :])
            nc.sync.dma_start(out=oT[hh * D:(hh + 1) * D, hp, :],
                              in_=ot_tmp[0:D, :])

        # Both heads' AV accumulate inline: the four [65, 512] accumulators
        # each fit a single PSUM bank, leaving the [128, 1024] score ring its
        # two 2-bank slots.
        for hp in range(H // 2):
            qh = [qkT[hh * D:(hh + 1) * D, hp, :] for hh in range(2)]
            kh = [qkT[hh * D:(hh + 1) * D, 6 + hp, :] for hh in range(2)]
            o_ps = [[pp.tile([D + 1, HALF], f32, tag="acc", bufs=4,
                             name=f"o_ps{hh}{nk}") for nk in range(2)]
                    for hh in range(2)]
            for mt in range(NT):
                pts = []
                for hh in range(2):
                    h = 2 * hp + hh
                    rpb_t = sb.tile([P, N], bf16, tag="rpb", bufs=4)
                    nc.sync.dma_start(out=rpb_t,
                                      in_=rpb_d[h, mt * P:(mt + 1) * P, :])
                    st = pp.tile([P, N], f32, tag="st", bufs=2, name="st")
                    for nk in range(2):
                        nc.tensor.matmul(st[:, NSL[nk]],
                                         lhsT=kh[hh][:, mt * P:(mt + 1) * P],
                                         rhs=qh[hh][:, NSL[nk]],
                                         start=True, stop=True)
                    ptile = sb.tile([P, N], bf16, tag="pt0", bufs=4)
                    pts.append(ptile)
                    nc.scalar.activation(out=ptile, in_=st, func=AF.Exp)
                    if has_mask:
                        # maskb holds 0/1 key-mask -> masked keys mult to 0
                        nc.vector.scalar_tensor_tensor(
                            out=ptile, in0=ptile, scalar=maskb[:, mt:mt + 1],
                            in1=rpb_t, op0=OP.mult, op1=OP.mult)
                    else:
                        nc.vector.tensor_mul(ptile, ptile, rpb_t)
                for hh in range(2):
                    for nk in range(2):
                        nc.tensor.matmul(o_ps[hh][nk][:, :],
                                         lhsT=vaug[:, mt, 2 * hp + hh, :],
                                         rhs=pts[hh][:, NSL[nk]],
                                         start=(mt == 0), stop=(mt == NT - 1))
            evac_head(o_ps[0], hp, 0)
            evac_head(o_ps[1], hp, 1)

        # ------- proj + residual 1, LN2 stats interleaved per chunk -------
        ln2_stt = ln_stats_alloc()
        ln2_xbs = []
        for mt in range(KC):
            wt = sb.tile([P, KC, P], bf16, tag="w6", bufs=3)
            nc.sync.dma_start(out=wt, in_=pw_d[mt])
            for nk in range(2):
                mm = pp.tile([P, HALF], f32, tag="acc", bufs=4)
                for ks in range(KC):
                    nc.tensor.matmul(mm, lhsT=wt[:, ks],
                                     rhs=oT[:, ks, NSL[nk]],
                                     start=(ks == 0), stop=(ks == KC - 1))
                if pb_nz:
                    nc.vector.tensor_scalar_add(mm, mm, vec(V_PB)[:, mt:mt + 1])
                nc.vector.scalar_tensor_tensor(
                    out=xres[:, mt, NSL[nk]], in0=mm,
                    scalar=vec(V_G1)[:, mt:mt + 1],
                    in1=xres[:, mt, NSL[nk]], op0=OP.mult, op1=OP.add)
            ln_stats_ch(ln2_stt, mt, ln2_xbs)

        # ---------------- LN2 finish, adapter-down, MLP ----------------
        xn2T = sb.tile([P, KC, N], bf16, tag="feat", bufs=2)
        ln_finish(ln2_stt, ln2_xbs, xn2T, V_L2G, V_L2B, ln2_triv)

        a1T = sb.tile([P, KR, N], bf16, tag="a1", bufs=1)
        for mt in range(KR):
            wt = sb.tile([P, KC, P], bf16, tag="w6", bufs=3)
            nc.sync.dma_start(out=wt, in_=ad_d[mt])
            for nk in range(2):
                mm = pp.tile([P, HALF], f32, tag="acc", bufs=4)
                for ks in range(KC):
                    nc.tensor.matmul(mm, lhsT=wt[:, ks],
                                     rhs=xn2T[:, ks, NSL[nk]],
                                     start=(ks == 0), stop=(ks == KC - 1))
                nc.scalar.activation(
                    out=a1T[:, mt, NSL[nk]], in_=mm, func=AF.Relu,
                    bias=(adb[:, mt:mt + 1] if adb_nz else zero_col))

        for nk in range(2):
            h1 = sb.tile([P, KM, HALF], bf16, tag="h1", bufs=1)
            for mt in range(KM):
                wt = sb.tile([P, KC, P], bf16, tag="w6", bufs=3)
                nc.sync.dma_start(out=wt, in_=f1_d[mt])
                mm = pp.tile([P, HALF], f32, tag="acc", bufs=4)
                for ks in range(KC):
                    nc.tensor.matmul(mm, lhsT=wt[:, ks],
                                     rhs=xn2T[:, ks, NSL[nk]],
                                     start=(ks == 0), stop=(ks == KC - 1))
                nc.scalar.activation(
                    out=h1[:, mt], in_=mm, func=AF.Gelu,
                    bias=(f1b[:, mt:mt + 1] if f1b_nz else zero_col))
            for mt in range(KC):
                w2 = sb.tile([P, KM, P], bf16, tag="w24", bufs=2)
                nc.sync.dma_start(out=w2, in_=f2_d[mt])
                au = sb.tile([P, KR, P], bf16, tag="w2", bufs=2)
                nc.sync.dma_start(out=au, in_=au_d[mt])
                mm = pp.tile([P, HALF], f32, tag="acc", bufs=4)
                for ks in range(KM):
                    nc.tensor.matmul(mm, lhsT=w2[:, ks], rhs=h1[:, ks],
                                     start=(ks == 0), stop=False)
                for ks in range(KR):
                    nc.tensor.matmul(mm, lhsT=au[:, ks],
                                     rhs=a1T[:, ks, NSL[nk]],
                                     start=False, stop=(ks == KR - 1))
                if fb_nz:
                    nc.vector.tensor_scalar_add(mm, mm, vec(V_FB)[:, mt:mt + 1])
                nc.vector.scalar_tensor_tensor(
                    out=xres[:, mt, NSL[nk]], in0=mm,
                    scalar=vec(V_G2)[:, mt:mt + 1],
                    in1=xres[:, mt, NSL[nk]], op0=OP.mult, op1=OP.add)
                nc.sync.dma_start(out=out_d[:, mt, NSL[nk]],
                                  in_=xres[:, mt, NSL[nk]])

    if not nc.is_finalized():
        nc.finalize()
    return nc


def _pack_w6(wT, km, kk):
    """[K, M] (K=contraction, M=out) -> [M//128, 128, K//128, 128] tiles
    laid out so each DMA partition read is contiguous."""
    K, M = wT.shape
    assert K == kk * P and M == km * P
    a = wT.reshape(kk, P, km, P)          # [ks, p, mt, col]
    return np.ascontiguousarray(a.transpose(2, 1, 0, 3)).astype(BF16)


def _stripe(v, k):
    """[k*128] -> [128, k] with v[ks*128+p] at [p, ks]."""
    return np.ascontiguousarray(v.reshape(k, P).T).astype(np.float32)


def prepare_core_inputs(x, mask, rpb, ln1_g, ln1_b, qkv_w, q_bias, v_bias,
                        proj_w, proj_b, gamma1, ln2_g, ln2_b, fc1_w, fc1_b,
                        fc2_w, fc2_b, ad_dw, ad_db, ad_uw, ad_ub, gamma2):
    """Host-side layout prep. Returns (shared_map, per_core_maps, flags)."""
    scale = D ** (-0.5)
    f32 = np.float32

    qkv_w = np.asarray(qkv_w, f32)
    wq = qkv_w[:C] * scale
    wk = qkv_w[C:2 * C]
    wv = qkv_w[2 * C:]
    wqkT = np.concatenate([wq, wk], 0).T          # [C, 1536]
    wqk = _pack_w6(wqkT, 12, KC)
    # wv used as matmul rhs: [p, ks, col] = wv[col, ks*128+p]
    wv_packed = np.ascontiguousarray(
        wv.T.reshape(KC, P, C).transpose(1, 0, 2)).astype(BF16)

    projw = _pack_w6(np.asarray(proj_w, f32).T, KC, KC)
    fc1w = _pack_w6(np.asarray(fc1_w, f32).T, KM, KC)
    fc2w = _pack_w6(np.asarray(fc2_w, f32).T, KC, KM)
    adw = _pack_w6(np.asarray(ad_dw, f32).T, KR, KC)
    auw = _pack_w6(np.asarray(ad_uw, f32).T, KC, KR)

    # exp(rpb) so the kernel can fold the bias into softmax's exp as a
    # multiply: exp(s + r) = exp(s) * exp(r)
    rpbT = np.ascontiguousarray(
        np.exp(np.asarray(rpb, f32).transpose(0, 2, 1))).astype(BF16)

    q_bias_s = np.asarray(q_bias, f32) * scale
    fb = np.asarray(fc2_b, f32) + np.asarray(ad_ub, f32)

    vecs = np.stack([
        _stripe(np.asarray(gamma1, f32), KC),
        _stripe(np.asarray(gamma2, f32), KC),
        _stripe(q_bias_s, KC),
        _stripe(np.asarray(proj_b, f32), KC),
        _stripe(fb, KC),
        _stripe(np.asarray(ln1_g, f32), KC),
        _stripe(np.asarray(ln1_b, f32), KC),
        _stripe(np.asarray(ln2_g, f32), KC),
        _stripe(np.asarray(ln2_b, f32), KC),
    ], 0)  # [NVEC, 128, KC]

    f1b = _stripe(np.asarray(fc1_b, f32), KM)
    adb = _stripe(np.asarray(ad_db, f32), KR)
    vb = np.asarray(v_bias, f32).reshape(1, C).astype(f32)

    mask = np.asarray(mask)
    has_mask = not bool(mask.all())

    flags = (
        has_mask,
        bool(np.any(q_bias_s)),
        bool(np.any(v_bias)),
        bool(np.any(proj_b)),
        bool(np.any(fc1_b)),
        bool(np.any(fb)),
        bool(np.any(ad_db)),
        bool(np.all(ln1_g == 1.0) and not np.any(ln1_b)),
        bool(np.all(ln2_g == 1.0) and not np.any(ln2_b)),
    )

    shared = {
        "rpbT": rpbT, "wqk": wqk, "wv": wv_packed, "projw": projw,
        "fc1w": fc1w, "fc2w": fc2w, "adw": adw, "auw": auw,
        "vecs": vecs, "fc1b": f1b, "adb": adb, "vbias": vb,
    }

    x = np.asarray(x, f32)
    per_core = []
    for b in range(B):
        xT = np.ascontiguousarray(
            x[b].T.reshape(KC, P, N).transpose(1, 0, 2)).astype(f32)
        if has_mask:
            mb = np.where(mask[b], 1.0, 0.0).astype(f32)    # [N] over keys m
            mb = np.ascontiguousarray(mb.reshape(NT, P).T)  # [128, NT]
        else:
            mb = np.zeros((P, NT), f32)
        m = dict(shared)
        m["xT"] = xT
        m["maskb"] = mb
        per_core.append(m)
    return per_core, flags


def _ensure_ntff_hook():
    """The agent image lacks ``antenv.axon_hooks``; provide it and register
    the ctypes NTFF profile hook so trace=True works under axon."""
    import types
    try:
        from antenv.axon_hooks import get_axon_ntff_profile_hook  # noqa: F401
        return
    except ImportError:
        pass
    import antenv
    mod = types.ModuleType("antenv.axon_hooks")
    _h = {"hook": None}
    mod.set_axon_ntff_profile_hook = lambda h: _h.__setitem__("hook", h)
    mod.get_axon_ntff_profile_hook = lambda: _h["hook"]
    sys.modules["antenv.axon_hooks"] = mod
    antenv.axon_hooks = mod
    try:
        from trn_agent_boot.trn_boot import _ntff_profile_via_ctypes
        hook = _ntff_profile_via_ctypes("/opt/axon/libaxon_pjrt.so")
        if hook is not None:
            mod.set_axon_ntff_profile_hook(hook)
    except Exception as e:  # profiling degrades, run still works
        print("ntff hook setup failed:", e)


def run_sharded(inputs, trace=False, trace_kwargs=None):
    """Compile (cached) + run on 8 cores. Returns (out [B,N,C] f32, results)."""
    from concourse.bass_utils import run_bass_kernel_spmd
    if trace:
        _ensure_ntff_hook()

    per_core, flags = prepare_core_inputs(**inputs)
    if flags not in _PROG_CACHE:
        _PROG_CACHE[flags] = _build(flags)
    nc = _PROG_CACHE[flags]

    kw = {}
    if trace:
        kw["trace"] = True
        kw["trace_cores"] = [0]
        if trace_kwargs:
            kw["trace_kwargs"] = trace_kwargs
    res = run_bass_kernel_spmd(nc, per_core, core_ids=list(range(B)), **kw)

    out = np.empty((B, N, C), np.float32)
    for b in range(B):
        oT = res.results[b]["outT"]          # [128, KC, N]
        out[b] = oT.transpose(1, 0, 2).reshape(C, N).T
    return out, res


def kernel(**inputs):
    out, _ = run_sharded(inputs, trace=False)
    return out


# revision 51
# speedup vs baseline: 1.0338x; 1.0338x over previous
"""Trainium2 Bass kernel for nn_Block_89361089561275 (dense transformer block).

Sharding: data-parallel over batch B=8 -> one batch element per NeuronCore.
No collectives. The whole block runs in "feature-transposed" layout
(features on SBUF partitions, tokens on the free dim), which makes every
matmul a natural lhsT/rhs pair with zero on-device transposes:

    xT [C, N] --LN1--> xnT --+--> qkT = wqk^T @ xn  [1536, N]   (q pre-scaled)
                             +--> v   = xn @ wv     [N, 768]    (token-major)
    per head h (heads 2j/2j+1 live at partition offsets 0/64 -> the K=64
    score matmuls auto-pack as 64x128 PE row tiles):
        ST[m, n] = k_h^T(d,m).T @ q_h^T(d,n)        (scores, transposed)
        P~ = exp(ST + rpb^T)                         (no max-subtraction;
                                                      scores are O(1) here)
        [colsum; o~^T] = [ones; v_h]^T @ P~          (ones column makes the
                                                      softmax denominator fall
                                                      out of the AV matmul)
        o^T = o~^T * (1/colsum)  broadcast over d
    x1 = x + gamma1 * (proj_w^T-matmul + b);  LN2;  MLP + adapter fused into
    one PSUM accumulation; out = x1 + gamma2 * (...).

Weights are transposed/packed/cast to bf16 on the host (layout prep only);
all matmul accumulation is fp32 in PSUM.
"""

import sys

for _p in ("/opt/trn_rl_repo",):
    if _p not in sys.path:
        sys.path.insert(0, _p)

import numpy as np
import ml_dtypes

BF16 = ml_dtypes.bfloat16

B, N, C, H = 8, 1024, 768, 12
D = C // H            # 64
MLP = 4 * C           # 3072
RED = C // 3          # 256
EPS = 1e-5
P = 128
KC = C // P           # 6   c-chunks
KM = MLP // P         # 24  mlp-chunks
KR = RED // P         # 2   adapter chunks
NT = N // P           # 8   token tiles
HALF = 512
NSL = (slice(0, HALF), slice(HALF, N))

_PROG_CACHE: dict = {}

# indices into the packed [n, 128, KC] per-feature vector table
V_G1, V_G2, V_QB, V_PB, V_FB, V_L1G, V_L1B, V_L2G, V_L2B = range(9)
NVEC = 9


def _build(flags):
    """Build the single-core Bass program. flags is a tuple of bools:
    (has_mask, qb_nz, vb_nz, pb_nz, f1b_nz, fb_nz, adb_nz,
     ln1_gb_triv, ln2_gb_triv)
    """
    (has_mask, qb_nz, vb_nz, pb_nz, f1b_nz, fb_nz, adb_nz,
     ln1_triv, ln2_triv) = flags

    import concourse.tile as tile
    from concourse import bacc, mybir
    from contextlib import ExitStack

    f32 = mybir.dt.float32
    bf16 = mybir.dt.bfloat16
    AF = mybir.ActivationFunctionType
    OP = mybir.AluOpType

    nc = bacc.Bacc("TRN2")

    # ---- external I/O ----
    x_d = nc.declare_dram_parameter("xT", [P, KC, N], f32, isOutput=False)
    rpb_d = nc.declare_dram_parameter("rpbT", [H, N, N], bf16, isOutput=False)
    wqk_d = nc.declare_dram_parameter("wqk", [12, P, KC, P], bf16, isOutput=False)
    wv_d = nc.declare_dram_parameter("wv", [P, KC, C], bf16, isOutput=False)
    pw_d = nc.declare_dram_parameter("projw", [KC, P, KC, P], bf16, isOutput=False)
    f1_d = nc.declare_dram_parameter("fc1w", [KM, P, KC, P], bf16, isOutput=False)
    f2_d = nc.declare_dram_parameter("fc2w", [KC, P, KM, P], bf16, isOutput=False)
    ad_d = nc.declare_dram_parameter("adw", [KR, P, KC, P], bf16, isOutput=False)
    au_d = nc.declare_dram_parameter("auw", [KC, P, KR, P], bf16, isOutput=False)
    vec_d = nc.declare_dram_parameter("vecs", [NVEC, P, KC], f32, isOutput=False)
    f1b_d = nc.declare_dram_parameter("fc1b", [P, KM], f32, isOutput=False)
    adb_d = nc.declare_dram_parameter("adb", [P, KR], f32, isOutput=False)
    vb_d = nc.declare_dram_parameter("vbias", [1, C], f32, isOutput=False)
    mb_d = nc.declare_dram_parameter("maskb", [P, NT], f32, isOutput=False)
    out_d = nc.declare_dram_parameter("outT", [P, KC, N], f32, isOutput=True)

    with tile.TileContext(nc) as tc, ExitStack() as ctx:
        sb = ctx.enter_context(tc.tile_pool(name="sb", bufs=1))
        pp = ctx.enter_context(tc.tile_pool(name="pp", bufs=1, space="PSUM"))
        dram = ctx.enter_context(tc.tile_pool(name="dram", bufs=2, space="DRAM"))

        def broadcast(dst, src):
            """dst [p, n] sbuf <- src [1, n] sbuf replicated across partitions
            (via a DRAM bounce; SBUF APs cannot broadcast the partition dim)."""
            scratch = dram.tile([1, src.shape[-1]], src.dtype,
                                tag="bscratch", bufs=2, name="bscratch")
            nc.sync.dma_start(out=scratch, in_=src)
            nc.sync.dma_start(out=dst, in_=scratch.to_broadcast(dst.shape))

        def bcast_from_p128(dst, src128):
            """dst [p, n] sbuf <- broadcast of an [128, n/128] scattered row
            (element n lives at src128[n // (n/128), n % ...] p-major order)."""
            n = dst.shape[-1]
            scratch = dram.tile([1, n], src128.dtype,
                                tag="bscratch", bufs=2, name="bscratch")
            nc.sync.dma_start(out=scratch, in_=src128)
            nc.sync.dma_start(out=dst, in_=scratch.to_broadcast(dst.shape))

        # ---- persistent tiles ----
        xres = sb.tile([P, KC, N], f32, tag="xres", bufs=1)
        qkT = sb.tile([P, 12, N], bf16, tag="qkT", bufs=1)
        vaug = sb.tile([P, NT, H, D + 1], bf16, tag="vaug", bufs=1)
        ones_bf = sb.tile([P, 1], bf16, tag="ones", bufs=1)

        for ch in range(KC):  # per-chunk loads so LN1 stats start early
            nc.sync.dma_start(out=xres[:, ch], in_=x_d[:, ch])
        nc.vector.memset(ones_bf, 1.0)
        nc.vector.memset(vaug, 1.0)

        zero_col = sb.tile([P, 1], f32, tag="zcol", bufs=1)
        nc.vector.memset(zero_col, 0.0)
        eps_col = sb.tile([P, 1], f32, tag="ecol", bufs=1)
        nc.vector.memset(eps_col, float(EPS))

        vecs = sb.tile([P, NVEC, KC], f32, tag="vecs", bufs=1)
        nc.sync.dma_start(out=vecs, in_=vec_d[:].rearrange("v p k -> p v k"))

        def vec(i):
            return vecs[:, i]  # [128, KC]

        if f1b_nz:
            f1b = sb.tile([P, KM], f32, tag="f1b", bufs=1)
            nc.sync.dma_start(out=f1b, in_=f1b_d[:])
        if adb_nz:
            adb = sb.tile([P, KR], f32, tag="adb", bufs=1)
            nc.sync.dma_start(out=adb, in_=adb_d[:])
        if vb_nz:
            vb1 = sb.tile([1, C], f32, tag="vb1", bufs=1)
            nc.sync.dma_start(out=vb1, in_=vb_d[:])
            vb_b = sb.tile([P, C], f32, tag="vb_b", bufs=1)
            broadcast(vb_b, vb1)
        if has_mask:
            maskb = sb.tile([P, NT], f32, tag="maskb", bufs=1)
            nc.sync.dma_start(out=maskb, in_=mb_d[:])

        # ---------------- layernorm (feature-transposed) ----------------
        NP8 = N // P  # 8 — scattered-stat free width

        # LN stats live in one "st"-tag PSUM tile (idle outside attention):
        # sum(x) accumulates on row 0, sum(x^2) on row 32 (legal matmul
        # output base partitions), each token half on its own bank.
        def ln_stats_alloc():
            return pp.tile([33, N], f32, tag="st", bufs=2, name="ln_stats")

        def ln_stats_ch(stt, ch, xbs):
            xb = sb.tile([P, N], bf16, tag="xb", bufs=KC)
            xbs.append(xb)
            nc.vector.tensor_copy(out=xb, in_=xres[:, ch])
            x2 = sb.tile([P, N], bf16, tag="rpb", bufs=4)
            nc.vector.tensor_mul(x2, xb, xb)
            for nk in range(2):
                nc.tensor.matmul(stt[0:1, NSL[nk]], lhsT=ones_bf,
                                 rhs=xb[:, NSL[nk]],
                                 start=(ch == 0), stop=(ch == KC - 1))
                nc.tensor.matmul(stt[32:33, NSL[nk]], lhsT=ones_bf,
                                 rhs=x2[:, NSL[nk]],
                                 start=(ch == 0), stop=(ch == KC - 1))

        def ln_finish(stt, xbs, dst, g_i, b_i, triv):
            stat2 = sb.tile([33, N], f32, tag="stat", bufs=3)
            nc.vector.tensor_scalar_mul(stat2[0:1, :], stt[0:1, :], 1.0 / C)
            nc.vector.tensor_scalar_mul(stat2[32:33, :], stt[32:33, :],
                                        1.0 / C)
            # scatter the per-token stats across 128 partitions so the
            # var/sqrt/reciprocal chain runs on all lanes instead of one
            m128 = sb.tile([P, NP8], f32, tag="cs128", bufs=4)
            q128 = sb.tile([P, NP8], f32, tag="cs128", bufs=4)
            nc.sync.dma_start(out=m128, in_=stat2[0:1, :])
            nc.sync.dma_start(out=q128, in_=stat2[32:33, :])
            t128 = sb.tile([P, NP8], f32, tag="cs128", bufs=4)
            r128 = sb.tile([P, NP8], f32, tag="cs128", bufs=4)
            nc.vector.tensor_mul(t128, m128, m128)
            nc.vector.tensor_sub(q128, q128, t128)       # var
            nc.scalar.activation(q128, q128, AF.Sqrt, bias=eps_col)
            nc.vector.reciprocal(r128, q128)             # rstd
            # m128 <- -mean*rstd
            nc.vector.scalar_tensor_tensor(out=m128, in0=m128, scalar=-1.0,
                                           in1=r128, op0=OP.mult, op1=OP.mult)
            r_bf = sb.tile([P, NP8], bf16, tag="cs_bf", bufs=2)
            n_bf = sb.tile([P, NP8], bf16, tag="cs_bf", bufs=2)
            nc.vector.tensor_copy(out=r_bf, in_=r128)
            nc.vector.tensor_copy(out=n_bf, in_=m128)
            a_b = sb.tile([P, N], bf16, tag="bcast", bufs=2)
            c_b = sb.tile([P, N], bf16, tag="bcast", bufs=2)
            bcast_from_p128(a_b, r_bf)
            bcast_from_p128(c_b, n_bf)
            for ch in range(KC):
                t1 = sb.tile([P, N], bf16, tag="rpb", bufs=4)
                nc.vector.tensor_mul(t1, xbs[ch], a_b)
                if triv:
                    nc.vector.tensor_add(dst[:, ch], t1, c_b)
                else:
                    nc.vector.tensor_add(t1, t1, c_b)
                    nc.vector.tensor_scalar(
                        out=dst[:, ch], in0=t1,
                        scalar1=vec(g_i)[:, ch:ch + 1],
                        scalar2=vec(b_i)[:, ch:ch + 1],
                        op0=OP.mult, op1=OP.add)

        def layernorm(dst, g_i, b_i, triv):
            stt = ln_stats_alloc()
            xbs = []
            for ch in range(KC):
                ln_stats_ch(stt, ch, xbs)
            ln_finish(stt, xbs, dst, g_i, b_i, triv)

        # ---------------- LN1 + QKV ----------------
        xnT = sb.tile([P, KC, N], bf16, tag="feat", bufs=2)
        layernorm(xnT, V_L1G, V_L1B, ln1_triv)

        # q/k blocks interleaved so attention head-pair j can start as soon
        # as blocks j (q) and 6+j (k) are done
        for blk in (0, 6, 1, 7, 2, 8, 3, 9, 4, 10, 5, 11):
            wt = sb.tile([P, KC, P], bf16, tag="w6", bufs=3)
            nc.sync.dma_start(out=wt, in_=wqk_d[blk])
            for nk in range(2):
                mm = pp.tile([P, HALF], f32, tag="acc", bufs=4)
                for ks in range(KC):
                    nc.tensor.matmul(mm, lhsT=wt[:, ks],
                                     rhs=xnT[:, ks, NSL[nk]],
                                     start=(ks == 0), stop=(ks == KC - 1))
                dst = qkT[:, blk, NSL[nk]]
                if blk < 6 and qb_nz:
                    nc.vector.tensor_scalar_add(dst, mm, vec(V_QB)[:, blk:blk + 1])
                else:
                    nc.scalar.copy(out=dst, in_=mm)

        wv_sb = sb.tile([P, KC, C], bf16, tag="wv", bufs=1)
        nc.sync.dma_start(out=wv_sb, in_=wv_d[:])
        for t in range(NT):
            for off, cw in ((0, HALF), (HALF, C - HALF)):
                mm = pp.tile([P, HALF], f32, tag="acc", bufs=4)
                for ks in range(KC):
                    nc.tensor.matmul(mm[:, :cw],
                                     lhsT=xnT[:, ks, t * P:(t + 1) * P],
                                     rhs=wv_sb[:, ks, off:off + cw],
                                     start=(ks == 0), stop=(ks == KC - 1))
                dst = vaug[:, t, off // D:(off + cw) // D, :D]
                src = mm[:, :cw].rearrange("p (h d) -> p h d", d=D)
                if vb_nz:
                    nc.vector.tensor_add(
                        dst, src,
                        vb_b[:, off:off + cw].rearrange("p (h d) -> p h d", d=D))
                else:
                    nc.vector.tensor_copy(out=dst, in_=src)

        # ---------------- attention ----------------
        # Head pairs (2j, 2j+1) live at partition offsets 0/64 of qkT block j,
        # so their K=64 score matmuls execute concurrently in distinct PE
        # row-groups (and one head's LDWEIGHTS hides under the other's MM).
        oT = sb.tile([P, KC, N], bf16, tag="feat", bufs=2)

        def evac_head(o_ps, hp, hh):
            # copy the unnormalized head out of PSUM right away (frees the
            # accumulator slots for the next pair), normalize asynchronously
            ou = sb.tile([D + 1, N], f32, tag="stat", bufs=3)
            for nk in range(2):
                nc.vector.tensor_copy(out=ou[:, NSL[nk]], in_=o_ps[nk])
            # broadcast the raw colsum, then approx-reciprocal on all lanes
            # (the approx op's bit-twiddling seed is broken at base part 64)
            raw = sb.tile([P, N], f32, tag="rec", bufs=2)
            broadcast(raw, ou[D:D + 1, :])
            rb = sb.tile([P, N], f32, tag="bcast2", bufs=2)
            nc.vector.reciprocal_approx_fast(out=rb[0:D, :], in_=raw[0:D, :])
            ot_tmp = sb.tile([P, N], bf16, tag="ott", bufs=2)
            nc.vector.tensor_mul(ot_tmp[0:D, :], ou[0:D, :], rb[0:D, :])
            nc.sync.dma_start(out=oT[hh * D:(hh + 1) * D, hp, :],
                              in_=ot_tmp[0:D, :])

        # Both heads' AV accumulate inline: the four [65, 512] accumulators
        # each fit a single PSUM bank, leaving the [128, 1024] score ring its
        # two 2-bank slots.
        for hp in range(H // 2):
            qh = [qkT[hh * D:(hh + 1) * D, hp, :] for hh in range(2)]
            kh = [qkT[hh * D:(hh + 1) * D, 6 + hp, :] for hh in range(2)]
            o_ps = [[pp.tile([D + 1, HALF], f32, tag="acc", bufs=4,
                             name=f"o_ps{hh}{nk}") for nk in range(2)]
                    for hh in range(2)]
            for mt in range(NT):
                pts = []
                for hh in range(2):
                    h = 2 * hp + hh
                    rpb_t = sb.tile([P, N], bf16, tag="rpb", bufs=4)
                    nc.sync.dma_start(out=rpb_t,
                                      in_=rpb_d[h, mt * P:(mt + 1) * P, :])
                    st = pp.tile([P, N], f32, tag="st", bufs=2, name="st")
                    for nk in range(2):
                        nc.tensor.matmul(st[:, NSL[nk]],
                                         lhsT=kh[hh][:, mt * P:(mt + 1) * P],
                                         rhs=qh[hh][:, NSL[nk]],
                                         start=True, stop=True)
                    ptile = sb.tile([P, N], bf16, tag="pt0", bufs=4)
                    pts.append(ptile)
                    nc.scalar.activation(out=ptile, in_=st, func=AF.Exp)
                    if has_mask:
                        # maskb holds 0/1 key-mask -> masked keys mult to 0
                        nc.vector.scalar_tensor_tensor(
                            out=ptile, in0=ptile, scalar=maskb[:, mt:mt + 1],
                            in1=rpb_t, op0=OP.mult, op1=OP.mult)
                    else:
                        nc.vector.tensor_mul(ptile, ptile, rpb_t)
                for hh in range(2):
                    for nk in range(2):
                        nc.tensor.matmul(o_ps[hh][nk][:, :],
                                         lhsT=vaug[:, mt, 2 * hp + hh, :],
                                         rhs=pts[hh][:, NSL[nk]],
                                         start=(mt == 0), stop=(mt == NT - 1))
            evac_head(o_ps[0], hp, 0)
            evac_head(o_ps[1], hp, 1)

        # ------- proj + residual 1, LN2 stats interleaved per chunk -------
        ln2_stt = ln_stats_alloc()
        ln2_xbs = []
        for mt in range(KC):
            wt = sb.tile([P, KC, P], bf16, tag="w6", bufs=3)
            nc.sync.dma_start(out=wt, in_=pw_d[mt])
            for nk in range(2):
                mm = pp.tile([P, HALF], f32, tag="acc", bufs=4)
                for ks in range(KC):
                    nc.tensor.matmul(mm, lhsT=wt[:, ks],
                                     rhs=oT[:, ks, NSL[nk]],
                                     start=(ks == 0), stop=(ks == KC - 1))
                if pb_nz:
                    nc.vector.tensor_scalar_add(mm, mm, vec(V_PB)[:, mt:mt + 1])
                nc.vector.scalar_tensor_tensor(
                    out=xres[:, mt, NSL[nk]], in0=mm,
                    scalar=vec(V_G1)[:, mt:mt + 1],
                    in1=xres[:, mt, NSL[nk]], op0=OP.mult, op1=OP.add)
            ln_stats_ch(ln2_stt, mt, ln2_xbs)

        # ---------------- LN2 finish, adapter-down, MLP ----------------
        xn2T = sb.tile([P, KC, N], bf16, tag="feat", bufs=2)
        ln_finish(ln2_stt, ln2_xbs, xn2T, V_L2G, V_L2B, ln2_triv)

        a1T = sb.tile([P, KR, N], bf16, tag="a1", bufs=1)
        for mt in range(KR):
            wt = sb.tile([P, KC, P], bf16, tag="w6", bufs=3)
            nc.sync.dma_start(out=wt, in_=ad_d[mt])
            for nk in range(2):
                mm = pp.tile([P, HALF], f32, tag="acc", bufs=4)
                for ks in range(KC):
                    nc.tensor.matmul(mm, lhsT=wt[:, ks],
                                     rhs=xn2T[:, ks, NSL[nk]],
                                     start=(ks == 0), stop=(ks == KC - 1))
                nc.scalar.activation(
                    out=a1T[:, mt, NSL[nk]], in_=mm, func=AF.Relu,
                    bias=(adb[:, mt:mt + 1] if adb_nz else zero_col))

        for nk in range(2):
            h1 = sb.tile([P, KM, HALF], bf16, tag="h1", bufs=1)
            for mt in range(KM):
                wt = sb.tile([P, KC, P], bf16, tag="w6", bufs=3)
                nc.sync.dma_start(out=wt, in_=f1_d[mt])
                mm = pp.tile([P, HALF], f32, tag="acc", bufs=4)
                for ks in range(KC):
                    nc.tensor.matmul(mm, lhsT=wt[:, ks],
                                     rhs=xn2T[:, ks, NSL[nk]],
                                     start=(ks == 0), stop=(ks == KC - 1))
                nc.scalar.activation(
                    out=h1[:, mt], in_=mm, func=AF.Gelu,
                    bias=(f1b[:, mt:mt + 1] if f1b_nz else zero_col))
            for mt in range(KC):
                w2 = sb.tile([P, KM, P], bf16, tag="w24", bufs=2)
                nc.sync.dma_start(out=w2, in_=f2_d[mt])
                au = sb.tile([P, KR, P], bf16, tag="w2", bufs=2)
                nc.sync.dma_start(out=au, in_=au_d[mt])
                mm = pp.tile([P, HALF], f32, tag="acc", bufs=4)
                for ks in range(KM):
                    nc.tensor.matmul(mm, lhsT=w2[:, ks], rhs=h1[:, ks],
                                     start=(ks == 0), stop=False)
                for ks in range(KR):
                    nc.tensor.matmul(mm, lhsT=au[:, ks],
                                     rhs=a1T[:, ks, NSL[nk]],
                                     start=False, stop=(ks == KR - 1))
                if fb_nz:
                    nc.vector.tensor_scalar_add(mm, mm, vec(V_FB)[:, mt:mt + 1])
                nc.vector.scalar_tensor_tensor(
                    out=xres[:, mt, NSL[nk]], in0=mm,
                    scalar=vec(V_G2)[:, mt:mt + 1],
                    in1=xres[:, mt, NSL[nk]], op0=OP.mult, op1=OP.add)
                nc.sync.dma_start(out=out_d[:, mt, NSL[nk]],
                                  in_=xres[:, mt, NSL[nk]])

    if not nc.is_finalized():
        nc.finalize()
    return nc


def _pack_w6(wT, km, kk):
    """[K, M] (K=contraction, M=out) -> [M//128, 128, K//128, 128] tiles
    laid out so each DMA partition read is contiguous."""
    K, M = wT.shape
    assert K == kk * P and M == km * P
    a = wT.reshape(kk, P, km, P)          # [ks, p, mt, col]
    return np.ascontiguousarray(a.transpose(2, 1, 0, 3)).astype(BF16)


def _stripe(v, k):
    """[k*128] -> [128, k] with v[ks*128+p] at [p, ks]."""
    return np.ascontiguousarray(v.reshape(k, P).T).astype(np.float32)


def prepare_core_inputs(x, mask, rpb, ln1_g, ln1_b, qkv_w, q_bias, v_bias,
                        proj_w, proj_b, gamma1, ln2_g, ln2_b, fc1_w, fc1_b,
                        fc2_w, fc2_b, ad_dw, ad_db, ad_uw, ad_ub, gamma2):
    """Host-side layout prep. Returns (shared_map, per_core_maps, flags)."""
    scale = D ** (-0.5)
    f32 = np.float32

    qkv_w = np.asarray(qkv_w, f32)
    wq = qkv_w[:C] * scale
    wk = qkv_w[C:2 * C]
    wv = qkv_w[2 * C:]
    wqkT = np.concatenate([wq, wk], 0).T          # [C, 1536]
    wqk = _pack_w6(wqkT, 12, KC)
    # wv used as matmul rhs: [p, ks, col] = wv[col, ks*128+p]
    wv_packed = np.ascontiguousarray(
        wv.T.reshape(KC, P, C).transpose(1, 0, 2)).astype(BF16)

    projw = _pack_w6(np.asarray(proj_w, f32).T, KC, KC)
    fc1w = _pack_w6(np.asarray(fc1_w, f32).T, KM, KC)
    fc2w = _pack_w6(np.asarray(fc2_w, f32).T, KC, KM)
    adw = _pack_w6(np.asarray(ad_dw, f32).T, KR, KC)
    auw = _pack_w6(np.asarray(ad_uw, f32).T, KC, KR)

    # exp(rpb) so the kernel can fold the bias into softmax's exp as a
    # multiply: exp(s + r) = exp(s) * exp(r)
    rpbT = np.ascontiguousarray(
        np.exp(np.asarray(rpb, f32).transpose(0, 2, 1))).astype(BF16)

    q_bias_s = np.asarray(q_bias, f32) * scale
    fb = np.asarray(fc2_b, f32) + np.asarray(ad_ub, f32)

    vecs = np.stack([
        _stripe(np.asarray(gamma1, f32), KC),
        _stripe(np.asarray(gamma2, f32), KC),
        _stripe(q_bias_s, KC),
        _stripe(np.asarray(proj_b, f32), KC),
        _stripe(fb, KC),
        _stripe(np.asarray(ln1_g, f32), KC),
        _stripe(np.asarray(ln1_b, f32), KC),
        _stripe(np.asarray(ln2_g, f32), KC),
        _stripe(np.asarray(ln2_b, f32), KC),
    ], 0)  # [NVEC, 128, KC]

    f1b = _stripe(np.asarray(fc1_b, f32), KM)
    adb = _stripe(np.asarray(ad_db, f32), KR)
    vb = np.asarray(v_bias, f32).reshape(1, C).astype(f32)

    mask = np.asarray(mask)
    has_mask = not bool(mask.all())

    flags = (
        has_mask,
        bool(np.any(q_bias_s)),
        bool(np.any(v_bias)),
        bool(np.any(proj_b)),
        bool(np.any(fc1_b)),
        bool(np.any(fb)),
        bool(np.any(ad_db)),
        bool(np.all(ln1_g == 1.0) and not np.any(ln1_b)),
        bool(np.all(ln2_g == 1.0) and not np.any(ln2_b)),
    )

    shared = {
        "rpbT": rpbT, "wqk": wqk, "wv": wv_packed, "projw": projw,
        "fc1w": fc1w, "fc2w": fc2w, "adw": adw, "auw": auw,
        "vecs": vecs, "fc1b": f1b, "adb": adb, "vbias": vb,
    }

    x = np.asarray(x, f32)
    per_core = []
    for b in range(B):
        xT = np.ascontiguousarray(
            x[b].T.reshape(KC, P, N).transpose(1, 0, 2)).astype(f32)
        if has_mask:
            mb = np.where(mask[b], 1.0, 0.0).astype(f32)    # [N] over keys m
            mb = np.ascontiguousarray(mb.reshape(NT, P).T)  # [128, NT]
        else:
            mb = np.zeros((P, NT), f32)
        m = dict(shared)
        m["xT"] = xT
        m["maskb"] = mb
        per_core.append(m)
    return per_core, flags


def _ensure_ntff_hook():
    """The agent image lacks ``antenv.axon_hooks``; provide it and register
    the ctypes NTFF profile hook so trace=True works under axon."""
    import types
    try:
        from antenv.axon_hooks import get_axon_ntff_profile_hook  # noqa: F401
        return
    except ImportError:
        pass
    import antenv
    mod = types.ModuleType("antenv.axon_hooks")
    _h = {"hook": None}
    mod.set_axon_ntff_profile_hook = lambda h: _h.__setitem__("hook", h)
    mod.get_axon_ntff_profile_hook = lambda: _h["hook"]
    sys.modules["antenv.axon_hooks"] = mod
    antenv.axon_hooks = mod
    try:
        from trn_agent_boot.trn_boot import _ntff_profile_via_ctypes
        hook = _ntff_profile_via_ctypes("/opt/axon/libaxon_pjrt.so")
        if hook is not None:
            mod.set_axon_ntff_profile_hook(hook)
    except Exception as e:  # profiling degrades, run still works
        print("ntff hook setup failed:", e)


def run_sharded(inputs, trace=False, trace_kwargs=None):
    """Compile (cached) + run on 8 cores. Returns (out [B,N,C] f32, results)."""
    from concourse.bass_utils import run_bass_kernel_spmd
    if trace:
        _ensure_ntff_hook()

    per_core, flags = prepare_core_inputs(**inputs)
    if flags not in _PROG_CACHE:
        _PROG_CACHE[flags] = _build(flags)
    nc = _PROG_CACHE[flags]

    kw = {}
    if trace:
        kw["trace"] = True
        kw["trace_cores"] = [0]
        if trace_kwargs:
            kw["trace_kwargs"] = trace_kwargs
    res = run_bass_kernel_spmd(nc, per_core, core_ids=list(range(B)), **kw)

    out = np.empty((B, N, C), np.float32)
    for b in range(B):
        oT = res.results[b]["outT"]          # [128, KC, N]
        out[b] = oT.transpose(1, 0, 2).reshape(C, N).T
    return out, res


def kernel(**inputs):
    out, _ = run_sharded(inputs, trace=False)
    return out
